# revision 3
# baseline (speedup 1.0000x reference)
"""MoE CNN routing kernel for Trainium2 (8 NeuronCores, SPMD).

Strategy: the gate (tiny MLP) is computed on host in fp32; top-2 routing
determines which images each expert must process.  For expert e with n_e
routed images we compile a uniform SPMD program in which every core runs
expert e on C_e = ceil(n_e/8) images (capacity slots, zero-padded).  Images
are distributed round-robin over cores; experts with zero load are skipped
entirely.  The device computes raw per-expert class scores (fused
conv+BN+LeakyReLU chains in fp16 with fp32 PSUM accumulation, final
leaky+global-avg-pool fused into the PSUM evacuation); the host applies the
gate weights and scatters/sums into the final (32, 1000) output.
"""

import os
import sys
import math
import numpy as np

for _p in ("/opt/trn_rl_repo", "/root/.axon_site/_ro/trn_rl_repo"):
    if os.path.isdir(_p) and _p not in sys.path:
        sys.path.append(_p)

import concourse.bacc as bacc
import concourse.mybir as mybir
from concourse.tile import TileContext
from concourse.bass_utils import run_bass_kernel_spmd

FP16 = mybir.dt.float16
F32 = mybir.dt.float32
AF = mybir.ActivationFunctionType
ALU = mybir.AluOpType

EXPERT_CONFIGS = [
    [[32, 32], [64, 64], [128, 128]],
    [[64, 64], [128, 128], [256, 256]],
    [[48, 48], [96, 96], [192, 192]],
    [[64, 64, 64], [128, 128], [256]],
    [[96, 96], [192, 192], [384, 384]],
    [[64], [128, 128, 128], [256, 256]],
    [[80, 80], [160, 160], [320, 320]],
    [[64, 64], [128, 128, 128], [256]],
]
IN_C, IMG, NUM_CLASSES, BATCH = 3, 64, 1000, 32
N_CORES = 8
ALPHA, EPS = 0.1, 1e-5
BN_SCALE = 1.0 / math.sqrt(1.0 + EPS)   # gamma=1, var=1, mean=0, beta=0
M_OUT = 125                              # 1000 = 8 chunks of 125
N_MCHUNK = NUM_CLASSES // M_OUT


def _is_pw(stage_cfg, j):
    return len(stage_cfg) > 1 and (j % 2 == 0)


def _chunk128(c):
    """Split channel count into groups of <=128."""
    out = []
    while c > 0:
        g = min(128, c)
        out.append(g)
        c -= g
    return out


# ---------------------------------------------------------------------------
# Expert plan: pure function of the config; shared by host packer + builder.
# ---------------------------------------------------------------------------

class Unit:
    __slots__ = ("si", "j", "k", "cin", "cout", "res", "in_padded", "out_padded",
                 "wblocks", "first")

    def __init__(self, **kw):
        for k, v in kw.items():
            setattr(self, k, v)


class ExpertPlan:
    def __init__(self, cfg):
        self.cfg = cfg
        self.units = []
        res = IMG
        cin = IN_C
        units = []
        for si, stage_cfg in enumerate(cfg):
            for j, cout in enumerate(stage_cfg):
                k = 1 if _is_pw(stage_cfg, j) else 3
                units.append(Unit(si=si, j=j, k=k, cin=cin, cout=cout, res=res,
                                  in_padded=False, out_padded=False,
                                  wblocks=None, first=(si == 0 and j == 0)))
                cin = cout
            if si != 2:
                res //= 2
        # out conv as final unit (res is stage3 res = 16)
        units.append(Unit(si=3, j=0, k=1, cin=cin, cout=NUM_CLASSES, res=16,
                          in_padded=False, out_padded=False, wblocks=None,
                          first=False))
        # padding relationships: unit u's input buffer is padded iff u has k==3
        # (except the first unit, whose input comes from host: plain/im2col)
        for idx, u in enumerate(units):
            if u.k == 3 and not u.first:
                u.in_padded = True
        for idx, u in enumerate(units[:-1]):
            nxt = units[idx + 1]
            # padding across a stage boundary is produced by the maxpool,
            # not by the conv that precedes it
            u.out_padded = nxt.in_padded and (nxt.si == u.si)
        self.units = units
        # weight packing layout: col offsets in the [128, wcols] fp16 pack
        col = 0
        for u in units:
            blocks = []   # (tap, kg_index, kg_size, m_index, m_size, col_off, repl4)
            taps = u.k * u.k
            if u.first:
                # replicated at partition offsets 0/32/64/96; K = cin*taps (<=27)
                ksz = u.cin * taps
                assert ksz <= 32
                for m_i, m_sz in enumerate(_chunk128(u.cout)):
                    blocks.append((0, 0, ksz, m_i, m_sz, col, True))
                    col += m_sz
            else:
                kgs = _chunk128(u.cin)
                mts = _chunk128(u.cout) if u.si != 3 else [M_OUT] * N_MCHUNK
                for t in range(taps):
                    for kg_i, kg_sz in enumerate(kgs):
                        for m_i, m_sz in enumerate(mts):
                            blocks.append((t, kg_i, kg_sz, m_i, m_sz, col, False))
                            col += m_sz
            u.wblocks = blocks
        self.wcols = col

    def pack_weights(self, ep):
        """Pack expert params (numpy dict, as in setup_inputs) -> [128, wcols] fp16."""
        pack = np.zeros((128, self.wcols), np.float16)
        ui = 0
        for si, stage_cfg in enumerate(self.cfg):
            for j, _ in enumerate(stage_cfg):
                u = self.units[ui]
                w = np.asarray(ep['stages'][si][j]['w'], np.float32) * BN_SCALE
                # w: [cout, cin, k, k] -> per tap lhsT [cin, cout]
                self._place(pack, u, w)
                ui += 1
        u = self.units[-1]
        wo = np.asarray(ep['out_w'], np.float32)    # [1000, cin, 1, 1]
        self._place(pack, u, wo)
        return pack

    def _place(self, pack, u, w):
        cout, cin = w.shape[0], w.shape[1]
        if u.first:
            taps = u.k * u.k
            # lhsT rows: tap-major [tap*cin + ci] to match host im2col/x layout
            lhsT = np.transpose(w, (2, 3, 1, 0)).reshape(taps * cin, cout)
            for (_, _, ksz, m_i, m_sz, col, _) in u.wblocks:
                blk = lhsT[:, m_i * 128:m_i * 128 + m_sz]
                for q in range(4):
                    pack[32 * q:32 * q + ksz, col:col + m_sz] = blk.astype(np.float16)
        else:
            for (t, kg_i, kg_sz, m_i, m_sz, col, _) in u.wblocks:
                dy, dx = t // u.k, t % u.k
                m0 = m_i * (128 if u.si != 3 else M_OUT)
                blk = w[m0:m0 + m_sz, kg_i * 128:kg_i * 128 + kg_sz, dy, dx]
                pack[:kg_sz, col:col + m_sz] = blk.T.astype(np.float16)


PLANS = [ExpertPlan(cfg) for cfg in EXPERT_CONFIGS]


# ---------------------------------------------------------------------------
# Device program builder
# ---------------------------------------------------------------------------

def build_program(caps):
    """caps: tuple of 8 ints (images per core per expert). Returns (nc, meta)."""
    nc = bacc.Bacc("TRN2", target_bir_lowering=False, debug=False,
                   num_devices=N_CORES)
    active = [e for e in range(8) if caps[e] > 0]
    # DRAM I/O
    x_dram = {}
    w_dram = {}
    b_dram = {}
    for e in active:
        p = PLANS[e]
        u0 = p.units[0]
        krows = u0.cin * u0.k * u0.k          # 3 (1x1) or 27 (3x3 im2col)
        ncols = IMG * IMG                      # 4096
        x_dram[e] = nc.dram_tensor(f"x{e}", [caps[e], krows, ncols], FP16,
                                   kind="ExternalInput")
        w_dram[e] = nc.dram_tensor(f"w{e}", [128, p.wcols], FP16,
                                   kind="ExternalInput")
        b_dram[e] = nc.dram_tensor(f"b{e}", [M_OUT, N_MCHUNK], F32,
                                   kind="ExternalInput")
    out_cols = sum(N_MCHUNK * caps[e] for e in active)
    out_dram = nc.dram_tensor("out", [M_OUT, out_cols], F32,
                              kind="ExternalOutput")
    out_base = {}
    col = 0
    for e in active:
        out_base[e] = col
        col += N_MCHUNK * caps[e]

    with TileContext(nc) as tc:
        with tc.tile_pool(name="wp", bufs=2) as wpool, \
             tc.tile_pool(name="xp", bufs=2) as xpool, \
             tc.tile_pool(name="act", bufs=1) as apool, \
             tc.tile_pool(name="small", bufs=2) as spool, \
             tc.tile_pool(name="psum", bufs=8, space="PSUM") as pspool:
            for e in active:
                _build_expert(nc, tc, PLANS[e], caps[e],
                              x_dram[e], w_dram[e], b_dram[e],
                              out_dram, out_base[e],
                              wpool, xpool, apool, spool, pspool)
    nc.compile()
    return nc


def _evac_leaky(nc, pool, dst_ap, ps_ap, shape, use_act):
    """dst = leaky(ps); via ACT Prelu (1 op) or DVE 2-op."""
    if use_act:
        nc.scalar.activation(dst_ap, ps_ap, AF.Prelu, alpha=ALPHA)
    else:
        lt = pool.tile([shape[0], shape[1]], F32, tag="leaktmp")
        nc.vector.tensor_scalar_mul(lt[:, :], ps_ap, ALPHA)
        nc.vector.tensor_max(dst_ap, lt[:, :], ps_ap)


def _build_expert(nc, tc, plan, cap, x_d, w_d, b_d, out_d, out_base,
                  wpool, xpool, apool, spool, pspool):
    units = plan.units
    # weights for the whole expert: one tile, one DMA
    wt = wpool.tile([128, plan.wcols], FP16, tag="w")
    nc.sync.dma_start(wt[:, :], w_d[:, :])
    bt = spool.tile([M_OUT, N_MCHUNK], F32, tag="bias")
    nc.sync.dma_start(bt[:, :], b_d[:, :])
    acc = spool.tile([M_OUT, N_MCHUNK * cap], F32, tag="acc")

    for img in range(cap):
        # ---- load first-conv input (replicated at 4 partition offsets) ----
        u0 = units[0]
        krows = u0.cin * u0.k * u0.k
        xt = xpool.tile([128, IMG * IMG], FP16, tag="x")
        for q in range(4):
            nc.sync.dma_start(xt[32 * q:32 * q + krows, :], x_d[img, :, :])

        bufs = {}   # unit idx -> list of group tiles (output of that unit)

        for ui, u in enumerate(units):
            if u.si == 3:
                _build_out_conv(nc, plan, u, bufs[ui - 1], wt, bt, acc, img,
                                apool, pspool)
                break
            res = u.res
            pres = res + 2
            out_groups = _chunk128(u.cout)
            # allocate output tiles
            S = pres * pres if u.out_padded else res * res
            otiles = []
            for g_i, g_sz in enumerate(out_groups):
                t = apool.tile([g_sz, S], FP16, tag=f"s{u.si}u{u.j}g{g_i}")
                if u.out_padded:
                    # zero the one-pixel border (interior fully overwritten)
                    v = t[:, :].rearrange("c (h w) -> c h w", h=pres)
                    nc.gpsimd.memset(v[:, 0:1, :], 0.0)
                    nc.gpsimd.memset(v[:, pres - 1:pres, :], 0.0)
                    nc.gpsimd.memset(v[:, 1:pres - 1, 0:1], 0.0)
                    nc.gpsimd.memset(v[:, 1:pres - 1, pres - 1:pres], 0.0)
                otiles.append(t)
            use_act = (u.si == 0)   # ACT for stage-1 (big), DVE for stages 2/3

            if u.first:
                _build_first_conv(nc, u, xt, wt, otiles, pspool, apool, use_act)
            else:
                in_tiles = bufs[ui - 1]
                _build_conv(nc, u, in_tiles, wt, otiles, pspool, apool, use_act)

            # maxpool at stage end
            is_stage_end = (ui + 1 < len(units) and units[ui + 1].si != u.si
                            and u.si < 3 and units[ui + 1].si != 3)
            if is_stage_end and u.si < 2:
                nres = res // 2
                nxt = units[ui + 1]
                npad = nxt.in_padded
                npres = nres + 2
                ptiles = []
                for g_i, g_sz in enumerate(out_groups):
                    NS = npres * npres if npad else nres * nres
                    pt = apool.tile([g_sz, NS], FP16, tag=f"s{u.si}pg{g_i}")
                    if npad:
                        v = pt[:, :].rearrange("c (h w) -> c h w", h=npres)
                        nc.gpsimd.memset(v[:, 0:1, :], 0.0)
                        nc.gpsimd.memset(v[:, npres - 1:npres, :], 0.0)
                        nc.gpsimd.memset(v[:, 1:npres - 1, 0:1], 0.0)
                        nc.gpsimd.memset(v[:, 1:npres - 1, npres - 1:npres], 0.0)
                    src = otiles[g_i][:, :]
                    sv = src.rearrange("c (h w) -> c h w", h=res)
                    tmp = apool.tile([g_sz, res * nres], FP16, tag=f"pooltmp{g_i}")
                    nc.vector.tensor_max(tmp[:, :], sv[:, :, 0:res:2],
                                         sv[:, :, 1:res:2])
                    t3 = tmp[:, :].rearrange("c (h w) -> c h w", h=res)
                    if npad:
                        dst = pt[:, :].rearrange("c (h w) -> c h w", h=npres)[
                            :, 1:1 + nres, 1:1 + nres]
                    else:
                        dst = pt[:, :].rearrange("c (h w) -> c h w", h=nres)
                    nc.vector.tensor_max(dst, t3[:, 0:res:2, :], t3[:, 1:res:2, :])
                    ptiles.append(pt)
                bufs[ui] = ptiles
            else:
                bufs[ui] = otiles

        # DMA this image's accumulator columns happen after out conv (below)
    # one DMA for the whole expert's accumulator
    nc.sync.dma_start(out_d[:, out_base:out_base + N_MCHUNK * cap], acc[:, :])


def _interior(t, pres, r0, nrows, cols):
    v = t[:, :].rearrange("c (h w) -> c h w", h=pres)
    return v[:, 1 + r0:1 + r0 + nrows, 1:1 + cols]


def _build_first_conv(nc, u, xt, wt, otiles, pspool, apool, use_act):
    """First conv of the expert: K<=27 at 4 partition offsets, quartered."""
    res = u.res               # 64
    krows = u.cin * u.k * u.k
    (_, _, ksz, _, m_sz, col, _) = u.wblocks[0]
    assert len(u.wblocks) == 1 and m_sz == u.cout
    ot = otiles[0]
    pres = res + 2
    rows_per_q = res // 4     # 16
    tiles_per_q = rows_per_q // 8   # 2
    for q in range(4):
        for h in range(tiles_per_q):
            r0 = q * rows_per_q + h * 8
            ps = pspool.tile([u.cout, 8 * res], F32, tag="ps")
            rhs = xt[32 * q:32 * q + krows, :].rearrange(
                "c (h w) -> c h w", h=res)[:, r0:r0 + 8, :]
            lhs = wt[32 * q:32 * q + krows, col:col + m_sz]
            nc.tensor.matmul(ps[:, :], lhs, rhs, start=True, stop=True,
                             tile_position=(32 * q, 0))
            if u.out_padded:
                dst = _interior(ot, pres, r0, 8, res)
            else:
                dst = ot[:, :].rearrange("c (h w) -> c h w", h=res)[:, r0:r0 + 8, :]
            _evac_leaky(nc, apool, dst, ps[:, :], [u.cout, 8 * res], use_act)


def _build_conv(nc, u, in_tiles, wt, otiles, pspool, apool, use_act):
    """General conv unit (1x1 or 3x3) over padded/plain input groups."""
    res = u.res
    pres = res + 2
    taps = u.k * u.k
    kgs = _chunk128(u.cin)
    mts = _chunk128(u.cout)
    # spatial tiling: 8-row tiles at res>=32 (N = 8*res in {512, 256});
    # at res 16 one tile of 256
    rows_per_tile = 8 if res >= 32 else 16
    n_tiles = res // rows_per_tile
    # block lookup: (tap, kg, m) -> col
    bcol = {}
    for (t, kg_i, kg_sz, m_i, m_sz, col, _) in u.wblocks:
        bcol[(t, kg_i, m_i)] = (col, kg_sz, m_sz)
    for ti in range(n_tiles):
        r0 = ti * rows_per_tile
        for m_i, m_sz in enumerate(mts):
            ps = pspool.tile([m_sz, rows_per_tile * res], F32, tag="ps")
            n_acc = taps * len(kgs)
            ai = 0
            for t in range(taps):
                dy, dx = t // u.k, t % u.k
                for kg_i, kg_sz in enumerate(kgs):
                    col, ksz, msz = bcol[(t, kg_i, m_i)]
                    lhs = wt[:ksz, col:col + msz]
                    it = in_tiles[kg_i]
                    if u.in_padded:
                        v = it[:, :].rearrange("c (h w) -> c h w", h=pres)
                        rhs = v[:, r0 + dy:r0 + dy + rows_per_tile, dx:dx + res]
                    else:
                        v = it[:, :].rearrange("c (h w) -> c h w", h=res)
                        rhs = v[:, r0:r0 + rows_per_tile, :]
                    nc.tensor.matmul(ps[:, :], lhs, rhs,
                                     start=(ai == 0), stop=(ai == n_acc - 1))
                    ai += 1
            ot = otiles[m_i]
            if u.out_padded:
                dst = _interior(ot, pres, r0, rows_per_tile, res)
            else:
                dst = ot[:, :].rearrange("c (h w) -> c h w", h=res)[
                    :, r0:r0 + rows_per_tile, :]
            _evac_leaky(nc, apool, dst, ps[:, :], [m_sz, rows_per_tile * res],
                        use_act)


def _build_out_conv(nc, plan, u, in_tiles, wt, bt, acc, img, apool, pspool):
    """1x1 conv to 1000 classes + leaky + mean, fused via Prelu accum_out."""
    kgs = _chunk128(u.cin)
    S = 256   # 16*16
    bcol = {}
    for (t, kg_i, kg_sz, m_i, m_sz, col, _) in u.wblocks:
        bcol[(kg_i, m_i)] = (col, kg_sz, m_sz)
    for m_i in range(N_MCHUNK):
        ps = pspool.tile([M_OUT, S], F32, tag="ps")
        for kg_i, kg_sz in enumerate(kgs):
            col, ksz, msz = bcol[(kg_i, m_i)]
            nc.tensor.matmul(ps[:, :], wt[:ksz, col:col + msz],
                             in_tiles[kg_i][:, :],
                             start=(kg_i == 0), stop=(kg_i == len(kgs) - 1))
        scratch = apool.tile([M_OUT, S], FP16, tag="oscratch")
        nc.scalar.activation(scratch[:, :], ps[:, :], AF.Prelu,
                             bias=bt[:, m_i:m_i + 1], scale=1.0 / S,
                             alpha=ALPHA,
                             accum_out=acc[:, img * N_MCHUNK + m_i:
                                           img * N_MCHUNK + m_i + 1])


# ---------------------------------------------------------------------------
# Host side: gate, routing, packing, combine
# ---------------------------------------------------------------------------

_CACHE = {}


def _gate_host(x, gate):
    B = x.shape[0]
    pooled = x.reshape(B, IN_C, 4, IMG // 4, 4, IMG // 4).mean(axis=(3, 5))
    gi = pooled.reshape(B, -1).astype(np.float32)
    hdn = np.maximum(gi @ np.asarray(gate['w1'], np.float32)
                     + np.asarray(gate['b1'], np.float32), 0.0)
    logits = hdn @ np.asarray(gate['w2'], np.float32) \
        + np.asarray(gate['b2'], np.float32)
    ti = np.argsort(-logits, kind='stable', axis=1)[:, :2]
    tv = np.take_along_axis(logits, ti, axis=1)
    m = tv.max(axis=1, keepdims=True)
    eg = np.exp(tv - m)
    tg = eg / eg.sum(axis=1, keepdims=True)
    gates = np.zeros((B, 8), np.float32)
    np.put_along_axis(gates, ti, tg.astype(np.float32), axis=1)
    return gates


def _im2col27(xi):
    """xi: [3, 64, 64] fp32 -> [27, 4096] fp16 (3x3, pad 1), tap-major rows."""
    xp = np.zeros((IN_C, IMG + 2, IMG + 2), np.float32)
    xp[:, 1:IMG + 1, 1:IMG + 1] = xi
    rows = []
    for dy in range(3):
        for dx in range(3):
            rows.append(xp[:, dy:dy + IMG, dx:dx + IMG].reshape(IN_C, -1))
    return np.concatenate(rows, axis=0).astype(np.float16)


def kernel(x, params):
    x = np.asarray(x, np.float32)
    gates = _gate_host(x, params['gate'])
    counts = (gates > 0).sum(axis=0)
    caps = tuple(int(math.ceil(c / N_CORES)) for c in counts)
    active = [e for e in range(8) if caps[e] > 0]

    if caps not in _CACHE:
        _CACHE[caps] = build_program(caps)
    nc = _CACHE[caps]

    # slot assignment: expert e image i -> core i%8, slot i//8
    slot_map = {e: [[] for _ in range(N_CORES)] for e in active}
    for e in active:
        imgs = np.where(gates[:, e] > 0)[0]
        for i, b in enumerate(imgs):
            slot_map[e][i % N_CORES].append(int(b))

    # pack weights (cached per params id — params are fixed per process)
    wkey = id(params)
    wcache = _CACHE.setdefault('w', {})
    if wkey not in wcache:
        packs = {}
        for e in range(8):
            packs[e] = PLANS[e].pack_weights(params['experts'][e])
        bias = {}
        for e in range(8):
            b = np.asarray(params['experts'][e]['out_b'], np.float32) / 256.0
            bias[e] = b.reshape(N_MCHUNK, M_OUT).T.copy()
        wcache[wkey] = (packs, bias)
    packs, bias = wcache[wkey]

    in_maps = []
    for core in range(N_CORES):
        im = {}
        for e in active:
            p = PLANS[e]
            u0 = p.units[0]
            krows = u0.cin * u0.k * u0.k
            xa = np.zeros((caps[e], krows, IMG * IMG), np.float16)
            for s, b in enumerate(slot_map[e][core]):
                if krows == IN_C:
                    xa[s] = x[b].reshape(IN_C, -1).astype(np.float16)
                else:
                    xa[s] = _im2col27(x[b])
            im[f"x{e}"] = xa
            im[f"w{e}"] = packs[e]
            im[f"b{e}"] = bias[e]
        in_maps.append(im)

    trace = bool(os.environ.get("MOE_TRACE"))
    res = run_bass_kernel_spmd(nc, in_maps, core_ids=list(range(N_CORES)),
                               trace=trace)
    if trace:
        kernel.last_result = res

    out = np.zeros((BATCH, NUM_CLASSES), np.float32)
    out_base = {}
    col = 0
    for e in active:
        out_base[e] = col
        col += N_MCHUNK * caps[e]
    for e in active:
        for core in range(N_CORES):
            oc = res.results[core]["out"]
            for s, b in enumerate(slot_map[e][core]):
                y = oc[:, out_base[e] + s * N_MCHUNK:
                       out_base[e] + (s + 1) * N_MCHUNK]   # [125, 8]
                out[b] += gates[b, e] * y.T.reshape(-1)
    return out.astype(np.float32)


# revision 7
# speedup vs baseline: 1.0678x; 1.0678x over previous
"""MoE CNN routing kernel for Trainium2 (8 NeuronCores, SPMD).

Strategy: the gate (tiny MLP) is computed on host in fp32; top-2 routing
determines which images each expert must process.  For expert e with n_e
routed images we compile a uniform SPMD program in which every core runs
expert e on C_e = ceil(n_e/8) images (capacity slots, zero-padded).  Images
are distributed round-robin over cores; experts with zero load are skipped
entirely.  The device computes raw per-expert class scores (fused
conv+BN+LeakyReLU chains in fp16 with fp32 PSUM accumulation, final
leaky+global-avg-pool fused into the PSUM evacuation); the host applies the
gate weights and scatters/sums into the final (32, 1000) output.
"""

import os
import sys
import math
import numpy as np

for _p in ("/opt/trn_rl_repo", "/root/.axon_site/_ro/trn_rl_repo"):
    if os.path.isdir(_p) and _p not in sys.path:
        sys.path.append(_p)

import concourse.bacc as bacc
import concourse.mybir as mybir
from concourse.tile import TileContext
from concourse.bass_utils import run_bass_kernel_spmd

FP16 = mybir.dt.float16
F32 = mybir.dt.float32
AF = mybir.ActivationFunctionType
ALU = mybir.AluOpType

EXPERT_CONFIGS = [
    [[32, 32], [64, 64], [128, 128]],
    [[64, 64], [128, 128], [256, 256]],
    [[48, 48], [96, 96], [192, 192]],
    [[64, 64, 64], [128, 128], [256]],
    [[96, 96], [192, 192], [384, 384]],
    [[64], [128, 128, 128], [256, 256]],
    [[80, 80], [160, 160], [320, 320]],
    [[64, 64], [128, 128, 128], [256]],
]
IN_C, IMG, NUM_CLASSES, BATCH = 3, 64, 1000, 32
N_CORES = 8
ALPHA, EPS = 0.1, 1e-5
BN_SCALE = 1.0 / math.sqrt(1.0 + EPS)   # gamma=1, var=1, mean=0, beta=0
M_OUT = 125                              # 1000 = 8 chunks of 125
N_MCHUNK = NUM_CLASSES // M_OUT


def _is_pw(stage_cfg, j):
    return len(stage_cfg) > 1 and (j % 2 == 0)


def _chunk128(c):
    """Split channel count into groups of <=128."""
    out = []
    while c > 0:
        g = min(128, c)
        out.append(g)
        c -= g
    return out


# ---------------------------------------------------------------------------
# Expert plan: pure function of the config; shared by host packer + builder.
# ---------------------------------------------------------------------------

class Unit:
    __slots__ = ("si", "j", "k", "cin", "cout", "res", "in_padded", "out_padded",
                 "wblocks", "first")

    def __init__(self, **kw):
        for k, v in kw.items():
            setattr(self, k, v)


class ExpertPlan:
    def __init__(self, cfg):
        self.cfg = cfg
        self.units = []
        res = IMG
        cin = IN_C
        units = []
        for si, stage_cfg in enumerate(cfg):
            for j, cout in enumerate(stage_cfg):
                k = 1 if _is_pw(stage_cfg, j) else 3
                units.append(Unit(si=si, j=j, k=k, cin=cin, cout=cout, res=res,
                                  in_padded=False, out_padded=False,
                                  wblocks=None, first=(si == 0 and j == 0)))
                cin = cout
            if si != 2:
                res //= 2
        # out conv as final unit (res is stage3 res = 16)
        units.append(Unit(si=3, j=0, k=1, cin=cin, cout=NUM_CLASSES, res=16,
                          in_padded=False, out_padded=False, wblocks=None,
                          first=False))
        # padding relationships: unit u's input buffer is padded iff u has k==3
        # (except the first unit, whose input comes from host: plain/im2col)
        for idx, u in enumerate(units):
            if u.k == 3 and not u.first:
                u.in_padded = True
        for idx, u in enumerate(units[:-1]):
            nxt = units[idx + 1]
            # padding across a stage boundary is produced by the maxpool,
            # not by the conv that precedes it
            u.out_padded = nxt.in_padded and (nxt.si == u.si)
        self.units = units
        # weight packing layout: col offsets in the [128, wcols] fp16 pack
        col = 0
        for u in units:
            blocks = []   # (tap, kg_index, kg_size, m_index, m_size, col_off, repl4)
            taps = u.k * u.k
            if u.first:
                # replicated at partition offsets 0/32/64/96; K = cin*taps (<=27)
                ksz = u.cin * taps
                assert ksz <= 32
                for m_i, m_sz in enumerate(_chunk128(u.cout)):
                    blocks.append((0, 0, ksz, m_i, m_sz, col, True))
                    col += m_sz
            else:
                kgs = _chunk128(u.cin)
                mts = _chunk128(u.cout) if u.si != 3 else [M_OUT] * N_MCHUNK
                for t in range(taps):
                    for kg_i, kg_sz in enumerate(kgs):
                        for m_i, m_sz in enumerate(mts):
                            blocks.append((t, kg_i, kg_sz, m_i, m_sz, col, False))
                            col += m_sz
            u.wblocks = blocks
        self.wcols = col

    def pack_weights(self, ep):
        """Pack expert params (numpy dict, as in setup_inputs) -> [128, wcols] fp16."""
        pack = np.zeros((128, self.wcols), np.float16)
        ui = 0
        for si, stage_cfg in enumerate(self.cfg):
            for j, _ in enumerate(stage_cfg):
                u = self.units[ui]
                w = np.asarray(ep['stages'][si][j]['w'], np.float32) * BN_SCALE
                # w: [cout, cin, k, k] -> per tap lhsT [cin, cout]
                self._place(pack, u, w)
                ui += 1
        u = self.units[-1]
        wo = np.asarray(ep['out_w'], np.float32)    # [1000, cin, 1, 1]
        self._place(pack, u, wo)
        return pack

    def _place(self, pack, u, w):
        cout, cin = w.shape[0], w.shape[1]
        if u.first:
            taps = u.k * u.k
            # lhsT rows: tap-major [tap*cin + ci] to match host im2col/x layout
            lhsT = np.transpose(w, (2, 3, 1, 0)).reshape(taps * cin, cout)
            for (_, _, ksz, m_i, m_sz, col, _) in u.wblocks:
                blk = lhsT[:, m_i * 128:m_i * 128 + m_sz]
                for q in range(4):
                    pack[32 * q:32 * q + ksz, col:col + m_sz] = blk.astype(np.float16)
        else:
            for (t, kg_i, kg_sz, m_i, m_sz, col, _) in u.wblocks:
                dy, dx = t // u.k, t % u.k
                m0 = m_i * (128 if u.si != 3 else M_OUT)
                blk = w[m0:m0 + m_sz, kg_i * 128:kg_i * 128 + kg_sz, dy, dx]
                pack[:kg_sz, col:col + m_sz] = blk.T.astype(np.float16)


PLANS = [ExpertPlan(cfg) for cfg in EXPERT_CONFIGS]


# ---------------------------------------------------------------------------
# Device program builder
# ---------------------------------------------------------------------------

def build_program(caps):
    """caps: tuple of 8 ints (images per core per expert). Returns (nc, meta)."""
    nc = bacc.Bacc("TRN2", target_bir_lowering=False, debug=False,
                   num_devices=N_CORES)
    active = [e for e in range(8) if caps[e] > 0]
    # DRAM I/O
    x_dram = {}
    w_dram = {}
    b_dram = {}
    for e in active:
        p = PLANS[e]
        u0 = p.units[0]
        krows = u0.cin * u0.k * u0.k          # 3 (1x1) or 27 (3x3 im2col)
        ncols = IMG * IMG                      # 4096
        x_dram[e] = nc.dram_tensor(f"x{e}", [caps[e], krows, ncols], FP16,
                                   kind="ExternalInput")
        w_dram[e] = nc.dram_tensor(f"w{e}", [128, p.wcols], FP16,
                                   kind="ExternalInput")
        b_dram[e] = nc.dram_tensor(f"b{e}", [M_OUT, N_MCHUNK], F32,
                                   kind="ExternalInput")
    out_cols = sum(N_MCHUNK * caps[e] for e in active)
    out_dram = nc.dram_tensor("out", [M_OUT, out_cols], F32,
                              kind="ExternalOutput")
    out_base = {}
    col = 0
    for e in active:
        out_base[e] = col
        col += N_MCHUNK * caps[e]

    with TileContext(nc) as tc:
        with tc.tile_pool(name="wp", bufs=2) as wpool, \
             tc.tile_pool(name="xp", bufs=2) as xpool, \
             tc.tile_pool(name="act", bufs=1) as apool0, \
             tc.tile_pool(name="small", bufs=2) as spool, \
             tc.tile_pool(name="psum", bufs=8, space="PSUM") as pspool:
            apool = _PoolRouter(apool0)
            for e in active:
                _build_expert(nc, tc, PLANS[e], caps[e],
                              x_dram[e], w_dram[e], b_dram[e],
                              out_dram, out_base[e],
                              wpool, xpool, apool, spool, pspool)
    nc.compile()
    return nc


class _PoolRouter:
    """Single-buffer the big stage-0-resolution tiles; double-buffer the
    rest (cross-image pipelining) — chosen per tile via the bufs override."""

    def __init__(self, pool):
        self.pool = pool

    def tile(self, shape, dtype, tag=None):
        big = shape[1] * mybir.dt.size(dtype) > 6000
        return self.pool.tile(shape, dtype, tag=tag, name=tag,
                              bufs=(1 if big else 2))


def _evac_leaky(nc, pool, dst_ap, ps_ap, shape, use_act):
    """dst = leaky(ps); via ACT Prelu (1 op) or DVE 2-op."""
    if use_act:
        nc.scalar.activation(dst_ap, ps_ap, AF.Prelu, alpha=ALPHA)
    else:
        lt = pool.tile([128, 512], F32, tag="leaktmp")
        l = lt[:shape[0], :shape[1]]
        nc.vector.tensor_scalar_mul(l, ps_ap, ALPHA)
        nc.vector.tensor_max(dst_ap, l, ps_ap)


def _build_expert(nc, tc, plan, cap, x_d, w_d, b_d, out_d, out_base,
                  wpool, xpool, apool, spool, pspool):
    units = plan.units
    # weights for the whole expert: one tile, one DMA
    wt = wpool.tile([128, plan.wcols], FP16, tag="w")
    nc.sync.dma_start(wt[:, :], w_d[:, :])
    bt = spool.tile([M_OUT, N_MCHUNK], F32, tag="bias")
    nc.sync.dma_start(bt[:, :], b_d[:, :])
    acc = spool.tile([M_OUT, N_MCHUNK * cap], F32, tag="acc")

    for img in range(cap):
        # ---- load first-conv input (replicated at 4 partition offsets) ----
        u0 = units[0]
        krows = u0.cin * u0.k * u0.k
        xt = xpool.tile([128, IMG * IMG], FP16, tag="x")
        for q in range(4):
            nc.sync.dma_start(xt[32 * q:32 * q + krows, :], x_d[img, :, :])

        bufs = {}   # unit idx -> list of group tiles (output of that unit)

        for ui, u in enumerate(units):
            if u.si == 3:
                _build_out_conv(nc, plan, u, bufs[ui - 1], wt, bt, acc, img,
                                apool, pspool)
                break
            res = u.res
            pres = res + 2
            out_groups = _chunk128(u.cout)
            # allocate output tiles
            S = pres * pres if u.out_padded else res * res
            otiles = []
            for g_i, g_sz in enumerate(out_groups):
                t = apool.tile([g_sz, S], FP16, tag=f"s{u.si}u{u.j}g{g_i}")
                if u.out_padded:
                    # zero the one-pixel border (interior fully overwritten)
                    v = t[:, :].rearrange("c (h w) -> c h w", h=pres)
                    nc.gpsimd.memset(v[:, 0:1, :], 0.0)
                    nc.gpsimd.memset(v[:, pres - 1:pres, :], 0.0)
                    nc.gpsimd.memset(v[:, 1:pres - 1, 0:1], 0.0)
                    nc.gpsimd.memset(v[:, 1:pres - 1, pres - 1:pres], 0.0)
                otiles.append(t)
            if u.first:
                _build_first_conv(nc, u, xt, wt, otiles, pspool, apool)
            else:
                in_tiles = bufs[ui - 1]
                _build_conv(nc, u, in_tiles, wt, otiles, pspool, apool)

            # maxpool at stage end
            is_stage_end = (ui + 1 < len(units) and units[ui + 1].si != u.si
                            and u.si < 3 and units[ui + 1].si != 3)
            if is_stage_end and u.si < 2:
                nres = res // 2
                nxt = units[ui + 1]
                npad = nxt.in_padded
                npres = nres + 2
                ptiles = []
                for g_i, g_sz in enumerate(out_groups):
                    NS = npres * npres if npad else nres * nres
                    pt = apool.tile([g_sz, NS], FP16, tag=f"s{u.si}pg{g_i}")
                    if npad:
                        v = pt[:, :].rearrange("c (h w) -> c h w", h=npres)
                        nc.gpsimd.memset(v[:, 0:1, :], 0.0)
                        nc.gpsimd.memset(v[:, npres - 1:npres, :], 0.0)
                        nc.gpsimd.memset(v[:, 1:npres - 1, 0:1], 0.0)
                        nc.gpsimd.memset(v[:, 1:npres - 1, npres - 1:npres], 0.0)
                    src = otiles[g_i][:, :]
                    sv = src.rearrange("c (h w) -> c h w", h=res)
                    tmp = apool.tile([g_sz, res * nres], FP16, tag=f"pooltmp{g_i}")
                    nc.vector.tensor_max(tmp[:, :], sv[:, :, 0:res:2],
                                         sv[:, :, 1:res:2])
                    t3 = tmp[:, :].rearrange("c (h w) -> c h w", h=res)
                    if npad:
                        dst = pt[:, :].rearrange("c (h w) -> c h w", h=npres)[
                            :, 1:1 + nres, 1:1 + nres]
                    else:
                        dst = pt[:, :].rearrange("c (h w) -> c h w", h=nres)
                    nc.vector.tensor_max(dst, t3[:, 0:res:2, :], t3[:, 1:res:2, :])
                    ptiles.append(pt)
                bufs[ui] = ptiles
            else:
                bufs[ui] = otiles

        # DMA this image's accumulator columns happen after out conv (below)
    # one DMA for the whole expert's accumulator
    nc.sync.dma_start(out_d[:, out_base:out_base + N_MCHUNK * cap], acc[:, :])


def _interior(t, pres, r0, nrows, cols):
    v = t[:, :].rearrange("c (h w) -> c h w", h=pres)
    return v[:, 1 + r0:1 + r0 + nrows, 1:1 + cols]


def _build_first_conv(nc, u, xt, wt, otiles, pspool, apool):
    """First conv of the expert: K<=27 at 4 partition offsets, quartered."""
    res = u.res               # 64
    krows = u.cin * u.k * u.k
    (_, _, ksz, _, m_sz, col, _) = u.wblocks[0]
    assert len(u.wblocks) == 1 and m_sz == u.cout
    ot = otiles[0]
    pres = res + 2
    rows_per_q = res // 4     # 16
    tiles_per_q = rows_per_q // 8   # 2
    for q in range(4):
        for h in range(tiles_per_q):
            r0 = q * rows_per_q + h * 8
            ps = pspool.tile([u.cout, 8 * res], F32, tag="ps")
            rhs = xt[32 * q:32 * q + krows, :].rearrange(
                "c (h w) -> c h w", h=res)[:, r0:r0 + 8, :]
            lhs = wt[32 * q:32 * q + krows, col:col + m_sz]
            nc.tensor.matmul(ps[:, :], lhs, rhs, start=True, stop=True,
                             tile_position=(32 * q, 0))
            if u.out_padded:
                dst = _interior(ot, pres, r0, 8, res)
            else:
                dst = ot[:, :].rearrange("c (h w) -> c h w", h=res)[:, r0:r0 + 8, :]
            _evac_leaky(nc, apool, dst, ps[:, :], [u.cout, 8 * res],
                        (q * tiles_per_q + h) % 2 == 0)


def _build_conv(nc, u, in_tiles, wt, otiles, pspool, apool):
    """General conv unit (1x1 or 3x3) over padded/plain input groups."""
    res = u.res
    pres = res + 2
    taps = u.k * u.k
    kgs = _chunk128(u.cin)
    mts = _chunk128(u.cout)
    # spatial tiling: N=512 tiles where possible (res 64: 8 rows, res 32:
    # 16 rows); at res 16 a single N=256 tile per image
    rows_per_tile = min(res, 512 // res)
    n_tiles = res // rows_per_tile
    # block lookup: (tap, kg, m) -> col
    bcol = {}
    for (t, kg_i, kg_sz, m_i, m_sz, col, _) in u.wblocks:
        bcol[(t, kg_i, m_i)] = (col, kg_sz, m_sz)
    for ti in range(n_tiles):
        r0 = ti * rows_per_tile
        for m_i, m_sz in enumerate(mts):
            ps = pspool.tile([m_sz, rows_per_tile * res], F32, tag="ps")
            n_acc = taps * len(kgs)
            ai = 0
            for t in range(taps):
                dy, dx = t // u.k, t % u.k
                for kg_i, kg_sz in enumerate(kgs):
                    col, ksz, msz = bcol[(t, kg_i, m_i)]
                    lhs = wt[:ksz, col:col + msz]
                    it = in_tiles[kg_i]
                    if u.in_padded:
                        v = it[:, :].rearrange("c (h w) -> c h w", h=pres)
                        rhs = v[:, r0 + dy:r0 + dy + rows_per_tile, dx:dx + res]
                    else:
                        v = it[:, :].rearrange("c (h w) -> c h w", h=res)
                        rhs = v[:, r0:r0 + rows_per_tile, :]
                    nc.tensor.matmul(ps[:, :], lhs, rhs,
                                     start=(ai == 0), stop=(ai == n_acc - 1))
                    ai += 1
            ot = otiles[m_i]
            if u.out_padded:
                dst = _interior(ot, pres, r0, rows_per_tile, res)
            else:
                dst = ot[:, :].rearrange("c (h w) -> c h w", h=res)[
                    :, r0:r0 + rows_per_tile, :]
            _evac_leaky(nc, apool, dst, ps[:, :], [m_sz, rows_per_tile * res],
                        (ti * len(mts) + m_i) % 2 == 0)


def _build_out_conv(nc, plan, u, in_tiles, wt, bt, acc, img, apool, pspool):
    """1x1 conv to 1000 classes + leaky + mean, fused via Prelu accum_out."""
    kgs = _chunk128(u.cin)
    S = 256   # 16*16
    bcol = {}
    for (t, kg_i, kg_sz, m_i, m_sz, col, _) in u.wblocks:
        bcol[(kg_i, m_i)] = (col, kg_sz, m_sz)
    for m_i in range(N_MCHUNK):
        ps = pspool.tile([M_OUT, S], F32, tag="ps")
        for kg_i, kg_sz in enumerate(kgs):
            col, ksz, msz = bcol[(kg_i, m_i)]
            nc.tensor.matmul(ps[:, :], wt[:ksz, col:col + msz],
                             in_tiles[kg_i][:, :],
                             start=(kg_i == 0), stop=(kg_i == len(kgs) - 1))
        scratch = apool.tile([M_OUT, S], FP16, tag="oscratch")
        nc.scalar.activation(scratch[:, :], ps[:, :], AF.Prelu,
                             bias=bt[:, m_i:m_i + 1], scale=1.0 / S,
                             alpha=ALPHA,
                             accum_out=acc[:, img * N_MCHUNK + m_i:
                                           img * N_MCHUNK + m_i + 1])


# ---------------------------------------------------------------------------
# Host side: gate, routing, packing, combine
# ---------------------------------------------------------------------------

_CACHE = {}


def _gate_host(x, gate):
    B = x.shape[0]
    pooled = x.reshape(B, IN_C, 4, IMG // 4, 4, IMG // 4).mean(axis=(3, 5))
    gi = pooled.reshape(B, -1).astype(np.float32)
    hdn = np.maximum(gi @ np.asarray(gate['w1'], np.float32)
                     + np.asarray(gate['b1'], np.float32), 0.0)
    logits = hdn @ np.asarray(gate['w2'], np.float32) \
        + np.asarray(gate['b2'], np.float32)
    ti = np.argsort(-logits, kind='stable', axis=1)[:, :2]
    tv = np.take_along_axis(logits, ti, axis=1)
    m = tv.max(axis=1, keepdims=True)
    eg = np.exp(tv - m)
    tg = eg / eg.sum(axis=1, keepdims=True)
    gates = np.zeros((B, 8), np.float32)
    np.put_along_axis(gates, ti, tg.astype(np.float32), axis=1)
    return gates


def _im2col27(xi):
    """xi: [3, 64, 64] fp32 -> [27, 4096] fp16 (3x3, pad 1), tap-major rows."""
    xp = np.zeros((IN_C, IMG + 2, IMG + 2), np.float32)
    xp[:, 1:IMG + 1, 1:IMG + 1] = xi
    rows = []
    for dy in range(3):
        for dx in range(3):
            rows.append(xp[:, dy:dy + IMG, dx:dx + IMG].reshape(IN_C, -1))
    return np.concatenate(rows, axis=0).astype(np.float16)


def kernel(x, params):
    x = np.asarray(x, np.float32)
    gates = _gate_host(x, params['gate'])
    counts = (gates > 0).sum(axis=0)
    caps = tuple(int(math.ceil(c / N_CORES)) for c in counts)
    active = [e for e in range(8) if caps[e] > 0]

    if caps not in _CACHE:
        _CACHE[caps] = build_program(caps)
    nc = _CACHE[caps]

    # slot assignment: expert e image i -> core i%8, slot i//8
    slot_map = {e: [[] for _ in range(N_CORES)] for e in active}
    for e in active:
        imgs = np.where(gates[:, e] > 0)[0]
        for i, b in enumerate(imgs):
            slot_map[e][i % N_CORES].append(int(b))

    # pack weights (cached per params id — params are fixed per process)
    wkey = id(params)
    wcache = _CACHE.setdefault('w', {})
    if wkey not in wcache:
        packs = {}
        for e in range(8):
            packs[e] = PLANS[e].pack_weights(params['experts'][e])
        bias = {}
        for e in range(8):
            b = np.asarray(params['experts'][e]['out_b'], np.float32) / 256.0
            bias[e] = b.reshape(N_MCHUNK, M_OUT).T.copy()
        wcache[wkey] = (packs, bias)
    packs, bias = wcache[wkey]

    in_maps = []
    for core in range(N_CORES):
        im = {}
        for e in active:
            p = PLANS[e]
            u0 = p.units[0]
            krows = u0.cin * u0.k * u0.k
            xa = np.zeros((caps[e], krows, IMG * IMG), np.float16)
            for s, b in enumerate(slot_map[e][core]):
                if krows == IN_C:
                    xa[s] = x[b].reshape(IN_C, -1).astype(np.float16)
                else:
                    xa[s] = _im2col27(x[b])
            im[f"x{e}"] = xa
            im[f"w{e}"] = packs[e]
            im[f"b{e}"] = bias[e]
        in_maps.append(im)

    trace = bool(os.environ.get("MOE_TRACE"))
    res = run_bass_kernel_spmd(nc, in_maps, core_ids=list(range(N_CORES)),
                               trace=trace)
    if trace:
        kernel.last_result = res

    out = np.zeros((BATCH, NUM_CLASSES), np.float32)
    out_base = {}
    col = 0
    for e in active:
        out_base[e] = col
        col += N_MCHUNK * caps[e]
    for e in active:
        for core in range(N_CORES):
            oc = res.results[core]["out"]
            for s, b in enumerate(slot_map[e][core]):
                y = oc[:, out_base[e] + s * N_MCHUNK:
                       out_base[e] + (s + 1) * N_MCHUNK]   # [125, 8]
                out[b] += gates[b, e] * y.T.reshape(-1)
    return out.astype(np.float32)


# revision 9
# speedup vs baseline: 1.3185x; 1.2347x over previous
"""MoE CNN routing kernel for Trainium2 (8 NeuronCores, SPMD).

Strategy: the gate (tiny MLP) is computed on host in fp32; top-2 routing
determines which images each expert must process.  For expert e with n_e
routed images we compile a uniform SPMD program in which every core runs
expert e on C_e = ceil(n_e/8) images (capacity slots, zero-padded).  Images
are distributed round-robin over cores; experts with zero load are skipped
entirely.  The device computes raw per-expert class scores (fused
conv+BN+LeakyReLU chains in fp16 with fp32 PSUM accumulation, final
leaky+global-avg-pool fused into the PSUM evacuation); the host applies the
gate weights and scatters/sums into the final (32, 1000) output.
"""

import os
import sys
import math
import numpy as np

for _p in ("/opt/trn_rl_repo", "/root/.axon_site/_ro/trn_rl_repo"):
    if os.path.isdir(_p) and _p not in sys.path:
        sys.path.append(_p)

import concourse.bacc as bacc
import concourse.mybir as mybir
from concourse.tile import TileContext
from concourse.bass_utils import run_bass_kernel_spmd

FP16 = mybir.dt.float16
F32 = mybir.dt.float32
AF = mybir.ActivationFunctionType
ALU = mybir.AluOpType

EXPERT_CONFIGS = [
    [[32, 32], [64, 64], [128, 128]],
    [[64, 64], [128, 128], [256, 256]],
    [[48, 48], [96, 96], [192, 192]],
    [[64, 64, 64], [128, 128], [256]],
    [[96, 96], [192, 192], [384, 384]],
    [[64], [128, 128, 128], [256, 256]],
    [[80, 80], [160, 160], [320, 320]],
    [[64, 64], [128, 128, 128], [256]],
]
IN_C, IMG, NUM_CLASSES, BATCH = 3, 64, 1000, 32
N_CORES = 8
ALPHA, EPS = 0.1, 1e-5
BN_SCALE = 1.0 / math.sqrt(1.0 + EPS)   # gamma=1, var=1, mean=0, beta=0
M_OUT = 125                              # 1000 = 8 chunks of 125
N_MCHUNK = NUM_CLASSES // M_OUT


def _is_pw(stage_cfg, j):
    return len(stage_cfg) > 1 and (j % 2 == 0)


def _chunk128(c):
    """Split channel count into groups of <=128."""
    out = []
    while c > 0:
        g = min(128, c)
        out.append(g)
        c -= g
    return out


# ---------------------------------------------------------------------------
# Expert plan: pure function of the config; shared by host packer + builder.
# ---------------------------------------------------------------------------

class Unit:
    __slots__ = ("si", "j", "k", "cin", "cout", "res", "in_padded", "out_padded",
                 "wblocks", "first")

    def __init__(self, **kw):
        for k, v in kw.items():
            setattr(self, k, v)


class ExpertPlan:
    def __init__(self, cfg):
        self.cfg = cfg
        self.units = []
        res = IMG
        cin = IN_C
        units = []
        for si, stage_cfg in enumerate(cfg):
            for j, cout in enumerate(stage_cfg):
                k = 1 if _is_pw(stage_cfg, j) else 3
                units.append(Unit(si=si, j=j, k=k, cin=cin, cout=cout, res=res,
                                  in_padded=False, out_padded=False,
                                  wblocks=None, first=(si == 0 and j == 0)))
                cin = cout
            if si != 2:
                res //= 2
        # out conv as final unit (res is stage3 res = 16)
        units.append(Unit(si=3, j=0, k=1, cin=cin, cout=NUM_CLASSES, res=16,
                          in_padded=False, out_padded=False, wblocks=None,
                          first=False))
        # padding relationships: unit u's input buffer is padded iff u has k==3
        # (except the first unit, whose input comes from host: plain/im2col)
        for idx, u in enumerate(units):
            if u.k == 3 and not u.first:
                u.in_padded = True
        for idx, u in enumerate(units[:-1]):
            nxt = units[idx + 1]
            # padding across a stage boundary is produced by the maxpool,
            # not by the conv that precedes it
            u.out_padded = nxt.in_padded and (nxt.si == u.si)
        self.units = units
        # weight packing layout: col offsets in the [128, wcols] fp16 pack
        col = 0
        for u in units:
            blocks = []   # (tap, kg_index, kg_size, m_index, m_size, col_off, repl4)
            taps = u.k * u.k
            if u.first:
                # replicated at partition offsets 0/32/64/96; K = cin*taps (<=27)
                ksz = u.cin * taps
                assert ksz <= 32
                for m_i, m_sz in enumerate(_chunk128(u.cout)):
                    blocks.append((0, 0, ksz, m_i, m_sz, col, True))
                    col += m_sz
            else:
                kgs = _chunk128(u.cin)
                mts = _chunk128(u.cout) if u.si != 3 else [M_OUT] * N_MCHUNK
                for t in range(taps):
                    for kg_i, kg_sz in enumerate(kgs):
                        for m_i, m_sz in enumerate(mts):
                            blocks.append((t, kg_i, kg_sz, m_i, m_sz, col, False))
                            col += m_sz
            u.wblocks = blocks
        self.wcols = col

    def pack_weights(self, ep):
        """Pack expert params (numpy dict, as in setup_inputs) -> [128, wcols] fp16."""
        pack = np.zeros((128, self.wcols), np.float16)
        ui = 0
        for si, stage_cfg in enumerate(self.cfg):
            for j, _ in enumerate(stage_cfg):
                u = self.units[ui]
                w = np.asarray(ep['stages'][si][j]['w'], np.float32) * BN_SCALE
                # w: [cout, cin, k, k] -> per tap lhsT [cin, cout]
                self._place(pack, u, w)
                ui += 1
        u = self.units[-1]
        wo = np.asarray(ep['out_w'], np.float32)    # [1000, cin, 1, 1]
        self._place(pack, u, wo)
        return pack

    def _place(self, pack, u, w):
        cout, cin = w.shape[0], w.shape[1]
        if u.first:
            taps = u.k * u.k
            # lhsT rows: tap-major [tap*cin + ci] to match host im2col/x layout
            lhsT = np.transpose(w, (2, 3, 1, 0)).reshape(taps * cin, cout)
            for (_, _, ksz, m_i, m_sz, col, _) in u.wblocks:
                blk = lhsT[:, m_i * 128:m_i * 128 + m_sz]
                for q in range(4):
                    pack[32 * q:32 * q + ksz, col:col + m_sz] = blk.astype(np.float16)
        else:
            for (t, kg_i, kg_sz, m_i, m_sz, col, _) in u.wblocks:
                dy, dx = t // u.k, t % u.k
                m0 = m_i * (128 if u.si != 3 else M_OUT)
                blk = w[m0:m0 + m_sz, kg_i * 128:kg_i * 128 + kg_sz, dy, dx]
                pack[:kg_sz, col:col + m_sz] = blk.T.astype(np.float16)


PLANS = [ExpertPlan(cfg) for cfg in EXPERT_CONFIGS]


# ---------------------------------------------------------------------------
# Device program builder
# ---------------------------------------------------------------------------

class _PoolRouter:
    """Single-buffer the big stage-0-resolution tiles; double-buffer the
    rest (cross-job pipelining) — chosen per tile via the bufs override."""

    def __init__(self, pool):
        self.pool = pool

    def tile(self, shape, dtype, tag=None):
        big = shape[1] * mybir.dt.size(dtype) > 6000
        return self.pool.tile(shape, dtype, tag=tag, name=tag,
                              bufs=(1 if big else 2))


def _evac_leaky(nc, pool, dst_ap, ps_ap, shape, use_act):
    """dst = leaky(ps); via ACT Prelu (1 op) or DVE 2-op."""
    if use_act:
        nc.scalar.activation(dst_ap, ps_ap, AF.Prelu, alpha=ALPHA)
    else:
        lt = pool.tile([128, 512], F32, tag="leaktmp")
        l = lt[:shape[0], :shape[1]]
        nc.vector.tensor_scalar_mul(l, ps_ap, ALPHA)
        nc.vector.tensor_max(dst_ap, l, ps_ap)


def _make_pairs(caps):
    """Group (expert, img) jobs into skewed pairs. Singles go first (cold
    start anyway); prefer same-expert pairs, then big+small cross pairs."""
    jobs = {e: list(range(caps[e])) for e in range(8) if caps[e] > 0}
    pairs = []
    # same-expert pairs
    for e in sorted(jobs):
        while len(jobs[e]) >= 2:
            a = jobs[e].pop(0)
            b = jobs[e].pop(0)
            pairs.append(((e, a), (e, b)))
    singles = [(e, jobs[e][0]) for e in sorted(jobs) if jobs[e]]
    # pair remaining singles big-with-small by wcols
    singles.sort(key=lambda t: -PLANS[t[0]].wcols)
    cross = []
    while len(singles) >= 2:
        big = singles.pop(0)
        small = singles.pop()
        cross.append((big, small))
    leftovers = [(sgl, None) for sgl in singles]
    # order: leftovers (single) first, then cross pairs, then same-expert
    return leftovers + cross + pairs


def build_program(caps):
    """caps: tuple of 8 ints (images per core per expert)."""
    nc = bacc.Bacc("TRN2", target_bir_lowering=False, debug=False,
                   num_devices=N_CORES)
    active = [e for e in range(8) if caps[e] > 0]
    x_dram = {}
    w_dram = {}
    b_dram = {}
    for e in active:
        p = PLANS[e]
        u0 = p.units[0]
        krows = u0.cin * u0.k * u0.k
        x_dram[e] = nc.dram_tensor(f"x{e}", [caps[e], krows, IMG * IMG], FP16,
                                   kind="ExternalInput")
        w_dram[e] = nc.dram_tensor(f"w{e}", [128, p.wcols], FP16,
                                   kind="ExternalInput")
        b_dram[e] = nc.dram_tensor(f"b{e}", [M_OUT, N_MCHUNK], F32,
                                   kind="ExternalInput")
    out_cols = sum(N_MCHUNK * caps[e] for e in active)
    out_dram = nc.dram_tensor("out", [M_OUT, out_cols], F32,
                              kind="ExternalOutput")
    out_base = {}
    col = 0
    for e in active:
        out_base[e] = col
        col += N_MCHUNK * caps[e]

    pairs = _make_pairs(caps)
    with TileContext(nc) as tc:
        with tc.tile_pool(name="wp", bufs=2) as wpool, \
             tc.tile_pool(name="xp", bufs=2) as xpool, \
             tc.tile_pool(name="act", bufs=1) as apool0, \
             tc.tile_pool(name="small", bufs=2) as spool, \
             tc.tile_pool(name="psum", bufs=8, space="PSUM") as pspool:
            apool = _PoolRouter(apool0)
            pair_wcols = max(PLANS[a[0]].wcols
                             + (PLANS[b[0]].wcols if b is not None and b[0] != a[0] else 0)
                             for a, b in pairs)
            # per-expert accumulators + bias tiles (persist whole kernel)
            acc_t = {}
            bias_t = {}
            for e in active:
                acc_t[e] = spool.tile([M_OUT, N_MCHUNK * caps[e]], F32,
                                      tag=f"acc{e}", name=f"acc{e}", bufs=1)
                bias_t[e] = spool.tile([M_OUT, N_MCHUNK], F32,
                                       tag=f"bias{e}", name=f"bias{e}", bufs=1)
                nc.sync.dma_start(bias_t[e][:, :], b_dram[e][:, :])
            remaining = {e: caps[e] for e in active}

            for a, b in pairs:
                # pair weight tile: expert a's pack at col 0, b's after (if different)
                wt = wpool.tile([128, pair_wcols], FP16, tag="w", name="w")
                ea = a[0]
                nc.sync.dma_start(wt[:, :PLANS[ea].wcols], w_dram[ea][:, :])
                woff = {ea: 0}
                if b is not None and b[0] != ea:
                    eb = b[0]
                    woff[eb] = PLANS[ea].wcols
                    nc.sync.dma_start(
                        wt[:, woff[eb]:woff[eb] + PLANS[eb].wcols],
                        w_dram[eb][:, :])
                jobs = []
                for jb in (a, b):
                    if jb is None:
                        continue
                    e, img = jb
                    jobs.append(_Job(e, img, PLANS[e], x_dram[e], wt, woff[e],
                                     bias_t[e], acc_t[e]))
                _emit_pair(nc, jobs, xpool, apool, spool, pspool)
                for e, img in (a, b) if b is not None else (a,):
                    remaining[e] -= 1
                    if remaining[e] == 0:
                        nc.sync.dma_start(
                            out_d_slice(out_dram, out_base[e], caps[e]),
                            acc_t[e][:, :])
    nc.compile()
    return nc


def out_d_slice(out_dram, base, cap):
    return out_dram[:, base:base + N_MCHUNK * cap]


class _Job:
    def __init__(self, e, img, plan, x_d, wt, woff, bias_t, acc_t):
        self.e = e
        self.img = img
        self.plan = plan
        self.x_d = x_d
        self.wt = wt
        self.woff = woff
        self.bias_t = bias_t
        self.acc_t = acc_t
        self.bufs = {}
        self.xt = None


SKEW = 1


def _emit_pair(nc, jobs, xpool, apool, spool, pspool):
    """Emit the jobs' units interleaved with a SKEW-unit lag on job 2."""
    steps = []
    for ji, job in enumerate(jobs):
        n = len(job.plan.units)
        for k in range(n):
            steps.append((k + (SKEW * ji), ji, k))
    steps.sort(key=lambda t: (t[0], t[1]))
    for _, ji, k in steps:
        _emit_unit(nc, jobs[ji], k, xpool, apool, spool, pspool)


def _emit_unit(nc, job, k, xpool, apool, spool, pspool):
    units = job.plan.units
    u = units[k]
    if k == 0:
        u0 = units[0]
        krows = u0.cin * u0.k * u0.k
        job.xt = xpool.tile([128, IMG * IMG], FP16, tag="x", name="x")
        for q in range(4):
            nc.sync.dma_start(job.xt[32 * q:32 * q + krows, :],
                              job.x_d[job.img, :, :])
    if u.si == 3:
        _build_out_conv(nc, job.plan, u, job.bufs[k - 1], job.wt, job.woff,
                        job.bias_t, job.acc_t, job.img, apool, pspool)
        return
    res = u.res
    pres = res + 2
    out_groups = _chunk128(u.cout)
    S = pres * pres if u.out_padded else res * res
    otiles = []
    for g_i, g_sz in enumerate(out_groups):
        tg = f"s{u.si}u{u.j}g{g_i}"
        t = apool.tile([g_sz, S], FP16, tag=tg)
        if u.out_padded:
            v = t[:, :].rearrange("c (h w) -> c h w", h=pres)
            nc.gpsimd.memset(v[:, 0:1, :], 0.0)
            nc.gpsimd.memset(v[:, pres - 1:pres, :], 0.0)
            nc.gpsimd.memset(v[:, 1:pres - 1, 0:1], 0.0)
            nc.gpsimd.memset(v[:, 1:pres - 1, pres - 1:pres], 0.0)
        otiles.append(t)
    if u.first:
        _build_first_conv(nc, u, job.xt, job.wt, job.woff, otiles, pspool,
                          apool)
    else:
        _build_conv(nc, u, job.bufs[k - 1], job.wt, job.woff, otiles, pspool,
                    apool)
    # maxpool at stage end (stages 0 and 1)
    is_stage_end = (k + 1 < len(units) and units[k + 1].si != u.si
                    and u.si < 2)
    if is_stage_end:
        nres = res // 2
        nxt = units[k + 1]
        npad = nxt.in_padded
        npres = nres + 2
        ptiles = []
        for g_i, g_sz in enumerate(out_groups):
            NS = npres * npres if npad else nres * nres
            pt = apool.tile([g_sz, NS], FP16, tag=f"s{u.si}pg{g_i}")
            if npad:
                v = pt[:, :].rearrange("c (h w) -> c h w", h=npres)
                nc.gpsimd.memset(v[:, 0:1, :], 0.0)
                nc.gpsimd.memset(v[:, npres - 1:npres, :], 0.0)
                nc.gpsimd.memset(v[:, 1:npres - 1, 0:1], 0.0)
                nc.gpsimd.memset(v[:, 1:npres - 1, npres - 1:npres], 0.0)
            sv = otiles[g_i][:, :].rearrange("c (h w) -> c h w", h=res)
            # pool in two row-halves so the chain pipelines
            half = res // 2
            for hh in range(2):
                tmp = apool.tile([g_sz, half * nres], FP16,
                                 tag=f"pooltmp{g_i}")
                r0 = hh * half
                nc.vector.tensor_max(tmp[:, :],
                                     sv[:, r0:r0 + half, 0:res:2],
                                     sv[:, r0:r0 + half, 1:res:2])
                t3 = tmp[:, :].rearrange("c (h w) -> c h w", h=half)
                if npad:
                    dst = pt[:, :].rearrange("c (h w) -> c h w", h=npres)[
                        :, 1 + r0 // 2:1 + r0 // 2 + half // 2, 1:1 + nres]
                else:
                    dst = pt[:, :].rearrange("c (h w) -> c h w", h=nres)[
                        :, r0 // 2:r0 // 2 + half // 2, :]
                nc.vector.tensor_max(dst, t3[:, 0:half:2, :],
                                     t3[:, 1:half:2, :])
            ptiles.append(pt)
        job.bufs[k] = ptiles
    else:
        job.bufs[k] = otiles


def _interior(t, pres, r0, nrows, cols):
    v = t[:, :].rearrange("c (h w) -> c h w", h=pres)
    return v[:, 1 + r0:1 + r0 + nrows, 1:1 + cols]


def _build_first_conv(nc, u, xt, wt, woff, otiles, pspool, apool):
    """First conv of the expert: K<=27 at 4 partition offsets, quartered."""
    res = u.res               # 64
    krows = u.cin * u.k * u.k
    (_, _, ksz, _, m_sz, col, _) = u.wblocks[0]
    col += woff
    ot = otiles[0]
    pres = res + 2
    rows_per_q = res // 4     # 16
    tiles_per_q = rows_per_q // 8   # 2
    for q in range(4):
        for h in range(tiles_per_q):
            r0 = q * rows_per_q + h * 8
            ps = pspool.tile([u.cout, 8 * res], F32, tag="ps", name="ps")
            rhs = xt[32 * q:32 * q + krows, :].rearrange(
                "c (h w) -> c h w", h=res)[:, r0:r0 + 8, :]
            lhs = wt[32 * q:32 * q + krows, col:col + m_sz]
            nc.tensor.matmul(ps[:, :], lhs, rhs, start=True, stop=True,
                             tile_position=(32 * q, 0))
            if u.out_padded:
                dst = _interior(ot, pres, r0, 8, res)
            else:
                dst = ot[:, :].rearrange("c (h w) -> c h w", h=res)[:, r0:r0 + 8, :]
            _evac_leaky(nc, apool, dst, ps[:, :], [u.cout, 8 * res],
                        (q * tiles_per_q + h) % 2 == 0)


def _build_conv(nc, u, in_tiles, wt, woff, otiles, pspool, apool):
    """General conv unit (1x1 or 3x3) over padded/plain input groups."""
    res = u.res
    pres = res + 2
    taps = u.k * u.k
    kgs = _chunk128(u.cin)
    mts = _chunk128(u.cout)
    rows_per_tile = min(res, 512 // res)
    n_tiles = res // rows_per_tile
    bcol = {}
    for (t, kg_i, kg_sz, m_i, m_sz, col, _) in u.wblocks:
        bcol[(t, kg_i, m_i)] = (col + woff, kg_sz, m_sz)
    for ti in range(n_tiles):
        r0 = ti * rows_per_tile
        for m_i, m_sz in enumerate(mts):
            ps = pspool.tile([m_sz, rows_per_tile * res], F32, tag="ps",
                             name="ps")
            n_acc = taps * len(kgs)
            ai = 0
            for t in range(taps):
                dy, dx = t // u.k, t % u.k
                for kg_i, kg_sz in enumerate(kgs):
                    col, ksz, msz = bcol[(t, kg_i, m_i)]
                    lhs = wt[:ksz, col:col + msz]
                    it = in_tiles[kg_i]
                    if u.in_padded:
                        v = it[:, :].rearrange("c (h w) -> c h w", h=pres)
                        rhs = v[:, r0 + dy:r0 + dy + rows_per_tile, dx:dx + res]
                    else:
                        v = it[:, :].rearrange("c (h w) -> c h w", h=res)
                        rhs = v[:, r0:r0 + rows_per_tile, :]
                    nc.tensor.matmul(ps[:, :], lhs, rhs,
                                     start=(ai == 0), stop=(ai == n_acc - 1))
                    ai += 1
            ot = otiles[m_i]
            if u.out_padded:
                dst = _interior(ot, pres, r0, rows_per_tile, res)
            else:
                dst = ot[:, :].rearrange("c (h w) -> c h w", h=res)[
                    :, r0:r0 + rows_per_tile, :]
            _evac_leaky(nc, apool, dst, ps[:, :], [m_sz, rows_per_tile * res],
                        (ti * len(mts) + m_i) % 2 == 0)


def _build_out_conv(nc, plan, u, in_tiles, wt, woff, bt, acc, img, apool,
                    pspool):
    """1x1 conv to 1000 classes + leaky + mean, fused via Prelu accum_out."""
    kgs = _chunk128(u.cin)
    S = 256
    bcol = {}
    for (t, kg_i, kg_sz, m_i, m_sz, col, _) in u.wblocks:
        bcol[(kg_i, m_i)] = (col + woff, kg_sz, m_sz)
    for m_i in range(N_MCHUNK):
        ps = pspool.tile([M_OUT, S], F32, tag="ps", name="ps")
        for kg_i, kg_sz in enumerate(kgs):
            col, ksz, msz = bcol[(kg_i, m_i)]
            nc.tensor.matmul(ps[:, :], wt[:ksz, col:col + msz],
                             in_tiles[kg_i][:, :],
                             start=(kg_i == 0), stop=(kg_i == len(kgs) - 1))
        scratch = apool.tile([M_OUT, S], FP16, tag="oscratch")
        nc.scalar.activation(scratch[:, :], ps[:, :], AF.Prelu,
                             bias=bt[:, m_i:m_i + 1], scale=1.0 / S,
                             alpha=ALPHA,
                             accum_out=acc[:, img * N_MCHUNK + m_i:
                                           img * N_MCHUNK + m_i + 1])


# ---------------------------------------------------------------------------
# Host side: gate, routing, packing, combine
# ---------------------------------------------------------------------------

_CACHE = {}


def _gate_host(x, gate):
    B = x.shape[0]
    pooled = x.reshape(B, IN_C, 4, IMG // 4, 4, IMG // 4).mean(axis=(3, 5))
    gi = pooled.reshape(B, -1).astype(np.float32)
    hdn = np.maximum(gi @ np.asarray(gate['w1'], np.float32)
                     + np.asarray(gate['b1'], np.float32), 0.0)
    logits = hdn @ np.asarray(gate['w2'], np.float32) \
        + np.asarray(gate['b2'], np.float32)
    ti = np.argsort(-logits, kind='stable', axis=1)[:, :2]
    tv = np.take_along_axis(logits, ti, axis=1)
    m = tv.max(axis=1, keepdims=True)
    eg = np.exp(tv - m)
    tg = eg / eg.sum(axis=1, keepdims=True)
    gates = np.zeros((B, 8), np.float32)
    np.put_along_axis(gates, ti, tg.astype(np.float32), axis=1)
    return gates


def _im2col27(xi):
    """xi: [3, 64, 64] fp32 -> [27, 4096] fp16 (3x3, pad 1), tap-major rows."""
    xp = np.zeros((IN_C, IMG + 2, IMG + 2), np.float32)
    xp[:, 1:IMG + 1, 1:IMG + 1] = xi
    rows = []
    for dy in range(3):
        for dx in range(3):
            rows.append(xp[:, dy:dy + IMG, dx:dx + IMG].reshape(IN_C, -1))
    return np.concatenate(rows, axis=0).astype(np.float16)


def kernel(x, params):
    x = np.asarray(x, np.float32)
    gates = _gate_host(x, params['gate'])
    counts = (gates > 0).sum(axis=0)
    caps = tuple(int(math.ceil(c / N_CORES)) for c in counts)
    active = [e for e in range(8) if caps[e] > 0]

    if caps not in _CACHE:
        _CACHE[caps] = build_program(caps)
    nc = _CACHE[caps]

    # slot assignment: expert e image i -> core i%8, slot i//8
    slot_map = {e: [[] for _ in range(N_CORES)] for e in active}
    for e in active:
        imgs = np.where(gates[:, e] > 0)[0]
        for i, b in enumerate(imgs):
            slot_map[e][i % N_CORES].append(int(b))

    # pack weights (cached per params id — params are fixed per process)
    wkey = id(params)
    wcache = _CACHE.setdefault('w', {})
    if wkey not in wcache:
        packs = {}
        for e in range(8):
            packs[e] = PLANS[e].pack_weights(params['experts'][e])
        bias = {}
        for e in range(8):
            b = np.asarray(params['experts'][e]['out_b'], np.float32) / 256.0
            bias[e] = b.reshape(N_MCHUNK, M_OUT).T.copy()
        wcache[wkey] = (packs, bias)
    packs, bias = wcache[wkey]

    in_maps = []
    for core in range(N_CORES):
        im = {}
        for e in active:
            p = PLANS[e]
            u0 = p.units[0]
            krows = u0.cin * u0.k * u0.k
            xa = np.zeros((caps[e], krows, IMG * IMG), np.float16)
            for s, b in enumerate(slot_map[e][core]):
                if krows == IN_C:
                    xa[s] = x[b].reshape(IN_C, -1).astype(np.float16)
                else:
                    xa[s] = _im2col27(x[b])
            im[f"x{e}"] = xa
            im[f"w{e}"] = packs[e]
            im[f"b{e}"] = bias[e]
        in_maps.append(im)

    trace = bool(os.environ.get("MOE_TRACE"))
    res = run_bass_kernel_spmd(nc, in_maps, core_ids=list(range(N_CORES)),
                               trace=trace)
    if trace:
        kernel.last_result = res

    out = np.zeros((BATCH, NUM_CLASSES), np.float32)
    out_base = {}
    col = 0
    for e in active:
        out_base[e] = col
        col += N_MCHUNK * caps[e]
    for e in active:
        for core in range(N_CORES):
            oc = res.results[core]["out"]
            for s, b in enumerate(slot_map[e][core]):
                y = oc[:, out_base[e] + s * N_MCHUNK:
                       out_base[e] + (s + 1) * N_MCHUNK]   # [125, 8]
                out[b] += gates[b, e] * y.T.reshape(-1)
    return out.astype(np.float32)
